# revision 19
# baseline (speedup 1.0000x reference)
"""Two-layer tanh RNN (T=2048, B=64, I=H=256) on 8 Trainium2 NeuronCores.

Strategy
--------
Data-parallel over batch: each of the 8 cores gets B_local = 8 sequences and
runs the full T=2048 double-layer recurrence on-chip, no cross-core traffic.

Per core, everything is kept in a transposed layout (hidden dim on SBUF/PSUM
partitions, batch on the free dim) so the recurrent matmuls
    h_t = tanh(xp_t + W_hh @ h_{t-1})
need no per-step transposes:

  * The input GEMM xp0 = W_ih0 @ x_t (+bias, via a ones-row rank-1 matmul) is
    computed in bulk, 64 timesteps at a time, straight into a PSUM group
    (one 2-bank PSUM tile per 64 steps, double buffered).  The bias fold and
    bulk fill use N=512 moving operands, so they are cheap and are spread /
    slotted between the recurrence steps.
  * The serial recurrence accumulates 4 small matmuls (2 output halves x
    2 contraction halves, N=8) per step on top of the precomputed PSUM slice,
    then one ScalarE Tanh per step produces h_t^T (fp16) for the next step.
  * Layer 1 runs 64 steps behind layer 0 inside the same instruction stream:
    its input GEMM consumes layer 0's h1 window in bulk, and its recurrence +
    Tanh interleave with layer 0's on the same engines.
  * The final output is produced by one bulk Tanh per 64-step group reading
    the whole layer-1 PSUM tile and writing fp32 to SBUF staging, then DMA'd
    out.  (tanh is evaluated twice for layer 1; the bulk one is ~free.)

fp16 is used for all matmul operands (fp32 PSUM accumulation); measured
end-to-end L2 relative error vs the fp32 reference is ~4e-4.
"""

import sys

sys.path.insert(0, "/opt/trn_rl_repo")

import numpy as np

import concourse.bass as bass
import concourse.mybir as mybir
import concourse.tile as tile
from concourse import bacc
from concourse.bass_utils import run_bass_kernel_spmd

T_FULL = 2048
B_GLOBAL = 64
N_CORES = 8
B = B_GLOBAL // N_CORES  # 8 per core
H = 256
G = 64  # timesteps per PSUM group (64 steps * 8 batch = 512 = one bank)
FP16 = mybir.dt.float16
F32 = mybir.dt.float32
Tanh = mybir.ActivationFunctionType.Tanh


def _bulk_xp_mm(nc, ps, w_sb, rhs0, rhs1, bias_sb, ones_sb, m, idx, skip=True):
    """One of the 6 bulk input-GEMM matmuls for output half m.

    idx 0: contraction half 0 (start=True, clears the bank)
    idx 1: contraction half 1
    idx 2: bias fold via ones-row rank-1 matmul
    """
    out = ps[:, m, :]
    if idx == 0:
        nc.tensor.matmul(out, w_sb[:, 0, m * 128 : (m + 1) * 128], rhs0,
                         start=True, stop=False, skip_group_check=skip)
    elif idx == 1:
        nc.tensor.matmul(out, w_sb[:, 1, m * 128 : (m + 1) * 128], rhs1,
                         start=False, stop=False, skip_group_check=skip)
    else:
        nc.tensor.matmul(out, bias_sb[:1, m * 128 : (m + 1) * 128], ones_sb[:1, :],
                         start=False, stop=False, skip_group_check=skip)


def _chain_mms(nc, ps, w_sb, rhs_src, col, s):
    """The 4 recurrence matmuls of one timestep (N=8)."""
    for m in (0, 1):
        for k in (0, 1):
            nc.tensor.matmul(
                ps[:, m, 8 * s : 8 * s + 8],
                w_sb[:, k, m * 128 : (m + 1) * 128],
                rhs_src[:, k, col : col + 8],
                start=False,
                stop=(k == 1),
                skip_group_check=True,
            )


def _strip_same_engine_waits(nc):
    """Remove provably-redundant same-engine semaphore waits.

    Tile emits conservative WAW waits at tile granularity; when a wait targets
    the waiting instruction's own engine-completion semaphore and the required
    value is already guaranteed by queue order (engines dispatch and complete
    in strict FIFO), the wait is redundant.  Walrus rejects Activation
    instructions carrying more than one sync wait, so these must go.
    """
    for f in nc.m.functions:
        for bb in f.blocks:
            incs = {}  # (engine, sem id) -> total incs so far in queue order
            for ins in bb.instructions:
                si = ins.sync_info
                if si is None:
                    continue
                eng = str(ins.engine)
                waits = si.on_wait
                if waits and "Activation" in eng:
                    keep = []
                    for w in waits:
                        done = incs.get((eng, w.id), 0)
                        if (
                            w.wait_mode == "sem-ge-imm"
                            and w.wait_reg is None
                            and done >= w.wait_value
                        ):
                            continue
                        keep.append(w)
                    if len(keep) != len(waits):
                        si.on_wait = keep
                for u in si.on_update:
                    if u.update_mode == "sem-inc" and u.update_reg is None:
                        key = (eng, u.id)
                        incs[key] = incs.get(key, 0) + u.update_value


def build_nc(T=T_FULL, repeat=1):
    NG = T // G
    N = T * B  # free-dim length of x / out per half

    nc = bacc.Bacc(None)

    xT = nc.dram_tensor("xT", [2, 128, N], FP16, kind="ExternalInput")
    w_ih0 = nc.dram_tensor("w_ih0", [2, 128, 256], FP16, kind="ExternalInput")
    w_hh0 = nc.dram_tensor("w_hh0", [2, 128, 256], FP16, kind="ExternalInput")
    w_ih1 = nc.dram_tensor("w_ih1", [2, 128, 256], FP16, kind="ExternalInput")
    w_hh1 = nc.dram_tensor("w_hh1", [2, 128, 256], FP16, kind="ExternalInput")
    bias0 = nc.dram_tensor("bias0", [1, 256], FP16, kind="ExternalInput")
    bias1 = nc.dram_tensor("bias1", [1, 256], FP16, kind="ExternalInput")
    out = nc.dram_tensor("out", [2, 128, N], F32, kind="ExternalOutput")

    out_re = out[:, :, :].rearrange("a p n -> p a n")
    xT_re = xT[:, :, :].rearrange("a p n -> p a n")

    XCHUNK = 2048 if N % 2048 == 0 else 512
    n_xchunks = N // XCHUNK

    with tile.TileContext(nc) as tc:
        with (
            tc.tile_pool(name="consts", bufs=1) as consts,
            tc.tile_pool(name="xpool", bufs=1) as xpool,
            tc.tile_pool(name="h1pool", bufs=3) as h1pool,
            tc.tile_pool(name="h2pool", bufs=3) as h2pool,
            tc.tile_pool(name="outpool", bufs=2) as outpool,
            tc.tile_pool(name="ps0pool", bufs=2, space="PSUM") as ps0pool,
            tc.tile_pool(name="ps1pool", bufs=2, space="PSUM") as ps1pool,
        ):
            # --- constants / inputs ---
            w0i = consts.tile([128, 2, 256], FP16, tag="w0i")
            w0h = consts.tile([128, 2, 256], FP16, tag="w0h")
            w1i = consts.tile([128, 2, 256], FP16, tag="w1i")
            w1h = consts.tile([128, 2, 256], FP16, tag="w1h")
            b0 = consts.tile([1, 256], FP16, tag="b0")
            b1 = consts.tile([1, 256], FP16, tag="b1")
            ones = consts.tile([1, 512], FP16, tag="ones")

            nc.sync.dma_start(w0i[:, :, :], w_ih0[:, :, :].rearrange("a k m -> k a m"))
            nc.sync.dma_start(w0h[:, :, :], w_hh0[:, :, :].rearrange("a k m -> k a m"))
            nc.sync.dma_start(w1i[:, :, :], w_ih1[:, :, :].rearrange("a k m -> k a m"))
            nc.sync.dma_start(w1h[:, :, :], w_hh1[:, :, :].rearrange("a k m -> k a m"))
            nc.sync.dma_start(b0[:, :], bias0[:, :])
            nc.sync.dma_start(b1[:, :], bias1[:, :])
            nc.vector.memset(ones[:, :], 1.0)

            xs = []
            for c in range(n_xchunks):
                xt = xpool.tile([128, 2, XCHUNK], FP16, tag=f"x{c}")
                nc.sync.dma_start(
                    xt[:, :, :], xT_re[:, :, c * XCHUNK : (c + 1) * XCHUNK]
                )
                xs.append(xt)

            def x_rhs(g, k):
                """x^T [128, 512] moving operand for group g, contraction half k."""
                c, off = divmod(g * 512, XCHUNK)
                return xs[c][:, k, off : off + 512]

            for _rep in range(repeat):
                _pipeline(nc, T, NG, x_rhs, w0i, w0h, w1i, w1h, b0, b1, ones,
                          h1pool, h2pool, outpool, ps0pool, ps1pool, out_re)
    _strip_same_engine_waits(nc)
    nc.finalize()
    return nc


def _pipeline(nc, T, NG, x_rhs, w0i, w0h, w1i, w1h, b0, b1, ones,
              h1pool, h2pool, outpool, ps0pool, ps1pool, out_re):
    if True:
        if True:
            # --- pipeline state ---
            LAG = 80  # chain-1 trails chain-0 by this many slots
            h1_cur = h1_prev = None
            h2_cur = h2_prev = None
            ps1_cur = ps1_next = None

            # initial bulk fill of group 0 (layer 0)
            ps0_cur = ps0pool.tile([128, 2, 512], F32, tag="ps0")
            for m in (0, 1):
                for idx in (0, 1, 2):
                    _bulk_xp_mm(nc, ps0_cur, w0i, x_rhs(0, 0), x_rhs(0, 1),
                                b0, ones, m, idx)
            ps0_next = None

            spread0 = {8: (0, 0), 16: (0, 1), 24: (0, 2),
                       32: (1, 0), 40: (1, 1), 48: (1, 2)}
            spread1 = {0: (0, 0), 2: (0, 1), 4: (0, 2),
                       6: (1, 0), 8: (1, 1), 10: (1, 2)}

            for tau in range(T + LAG + 1):
                g, s = divmod(tau, G)

                # ---- layer 0 recurrence ----
                if tau < T:
                    if s == 0:
                        h1_cur = h1pool.tile([128, 2, 512], FP16, tag="h1")
                    if tau > 0:
                        src = h1_cur if s > 0 else h1_prev
                        col = 8 * (s - 1) if s > 0 else 504
                        _chain_mms(nc, ps0_cur, w0h, src, col, s)

                # ---- layer 1 bulk input GEMM, spread over early slots ----
                if G <= tau < T + G and s in spread1:
                    m, idx = spread1[s]
                    if s == 0:
                        ps1_next = ps1pool.tile([128, 2, 512], F32, tag="ps1")
                    hw = h1_prev if tau < T else h1_cur  # window g-1
                    _bulk_xp_mm(nc, ps1_next, w1i, hw[:, 0, :], hw[:, 1, :],
                                b1, ones, m, idx)
                    if s == 10:
                        ps1_ready = ps1_next

                # ---- spread next group's layer-0 bulk fill ----
                if tau < T and g + 1 < NG and s in spread0:
                    m, idx = spread0[s]
                    if s == 8:
                        ps0_next = ps0pool.tile([128, 2, 512], F32, tag="ps0")
                    _bulk_xp_mm(nc, ps0_next, w0i, x_rhs(g + 1, 0), x_rhs(g + 1, 1),
                                b0, ones, m, idx)

                # ---- layer 0 tanh ----
                if tau < T:
                    nc.scalar.activation(
                        h1_cur[:, :, 8 * s : 8 * s + 8],
                        ps0_cur[:, :, 8 * s : 8 * s + 8],
                        Tanh,
                    )

                # ---- layer 1 recurrence + tanh (LAG slots behind) ----
                if tau >= LAG and tau - LAG < T:
                    t1 = tau - LAG
                    g1, s1 = divmod(t1, G)
                    if s1 == 0:
                        h2_cur = h2pool.tile([128, 2, 512], FP16, tag="h2")
                        ps1_cur = ps1_ready
                    if t1 > 0:
                        src = h2_cur if s1 > 0 else h2_prev
                        col = 8 * (s1 - 1) if s1 > 0 else 504
                        _chain_mms(nc, ps1_cur, w1h, src, col, s1)
                    nc.scalar.activation(
                        h2_cur[:, :, 8 * s1 : 8 * s1 + 8],
                        ps1_cur[:, :, 8 * s1 : 8 * s1 + 8],
                        Tanh,
                    )
                    if s1 == G - 1:
                        out_t = outpool.tile([128, 2, 512], F32, tag="out")
                        nc.scalar.activation(out_t[:, :, :], ps1_cur[:, :, :], Tanh)
                        nc.sync.dma_start(
                            out_re[:, :, g1 * 512 : (g1 + 1) * 512], out_t[:, :, :]
                        )
                        h2_prev = h2_cur

                # ---- group-end bookkeeping ----
                if tau < T and s == G - 1:
                    h1_prev = h1_cur
                    ps0_cur = ps0_next


_NC_CACHE = {}


def _get_nc(T):
    if T not in _NC_CACHE:
        _NC_CACHE[T] = build_nc(T)
    return _NC_CACHE[T]


def _pack_inputs(x, W_ih0, W_hh0, b_ih0, b_hh0, W_ih1, W_hh1, b_ih1, b_hh1):
    x = np.asarray(x, dtype=np.float32)
    W_ih0, W_hh0 = np.asarray(W_ih0, np.float32), np.asarray(W_hh0, np.float32)
    W_ih1, W_hh1 = np.asarray(W_ih1, np.float32), np.asarray(W_hh1, np.float32)
    b_ih0, b_hh0 = np.asarray(b_ih0, np.float32), np.asarray(b_hh0, np.float32)
    b_ih1, b_hh1 = np.asarray(b_ih1, np.float32), np.asarray(b_hh1, np.float32)
    T = x.shape[0]
    wmaps = {
        "w_ih0": W_ih0, "w_hh0": W_hh0, "w_ih1": W_ih1, "w_hh1": W_hh1,
    }
    shared = {
        name: np.ascontiguousarray(w.T.reshape(2, 128, 256).astype(np.float16))
        for name, w in wmaps.items()
    }
    shared["bias0"] = np.ascontiguousarray(
        (b_ih0 + b_hh0).astype(np.float16).reshape(1, 256))
    shared["bias1"] = np.ascontiguousarray(
        (b_ih1 + b_hh1).astype(np.float16).reshape(1, 256))

    in_maps = []
    for c in range(N_CORES):
        xs = x[:, c * B : (c + 1) * B, :]  # [T, 8, 256]
        xt = np.ascontiguousarray(
            xs.transpose(2, 0, 1).reshape(2, 128, T * B).astype(np.float16))
        m = dict(shared)
        m["xT"] = xt
        in_maps.append(m)
    return in_maps


def _unpack_outputs(results, T):
    parts = []
    for c in range(N_CORES):
        o = results[c]["out"]  # [2, 128, T*B] f32
        h2 = o.reshape(2, 128, T, B).transpose(2, 3, 0, 1).reshape(T, B, 256)
        parts.append(h2)
    full = np.concatenate(parts, axis=1)  # [T, 64, 256]
    return np.ascontiguousarray(full.reshape(T * B_GLOBAL, 256).astype(np.float32))


def run(inputs, T=T_FULL, **spmd_kwargs):
    nc = _get_nc(T)
    in_maps = _pack_inputs(**inputs)
    res = run_bass_kernel_spmd(nc, in_maps, core_ids=list(range(N_CORES)),
                               **spmd_kwargs)
    return _unpack_outputs(res.results, T), res


def kernel(**inputs):
    out, _ = run(inputs)
    return out
